# revision 22
# baseline (speedup 1.0000x reference)
"""Trainium2 Bass kernel: fp8-emulated attention, 20 heads x 4096 x 64.

Sharding: flattened (head, q) rows split evenly across 8 cores ->
2.5 heads per core (2 full-head segments + 1 half-head segment each,
identical SPMD graph; per-core in_maps differ only in data).

Per-core algorithm (S.T layout flash-style, no max subtraction -- scores
are bounded ~|s|<7 so fp32 exp never overflows):
  q8 = fp8(q) replicated on both partition halves, d-major [128, 10240]
  k8 = fp8(k) pair-packed [128, 16*128] per head (even kv-blocks on
       partitions 0-63, odd on 64-127) -> row-packed (tile_position)
       pairs of QK^T matmuls, K=64 contraction each.
  S.T block [128 kv, 512 q] in PSUM -> exp split per half: ScalarE does
  cols 0:512 (exact exp, scale=1/8 fused), VectorE does cols 512:1024
  (Schraudolph int16 bit-trick -> bf16) on most pairs; a few pairs go
  all-ScalarE to balance engine load. The two halves live in different
  PSUM banks so ACT+DVE read PSUM concurrently (this split beats
  whole-pair alternation: it halves per-pair exp latency).
  PV: O.T[65, 512] += [V_blk | ones].T @ P.T_blk accumulated over 32
  kv-blocks in PSUM; row 64 = softmax denominators.
  Epilogue (split across two pair-slots of the next chunk): copy to
  SBUF, PE-transpose [65,128] tiles -> [128,65], DVE reciprocal of the
  denominator + ScalarE Copy-with-AP-scale -> out rows [q, 64] -> DMA.
The (chunk, pair) stream is a single flat pipeline with a 2-pair QK
lookahead that crosses chunk boundaries. Bulk input casts (fp32 ->
fp8/bf16) run on GpSimd ordered by first use; the startup-critical
first pieces are cast on DVE so the first QK starts ~13us in.
"""

import os

import numpy as np

import concourse.bass as bass
import concourse.tile as tile
from concourse import bacc, mybir
from concourse.bass import ts
from concourse.bass_utils import run_bass_kernel_spmd
from concourse.masks import make_identity

B, S, D = 20, 4096, 64
NCORES = 8
ROWS_PER_CORE = B * S // NCORES  # 10240
HALF = S // 2  # 2048
NQ = 512  # q columns per chunk (one PSUM bank of fp32)
NPAIR = 16  # kv-block pairs per head (32 blocks of 128)

F32 = mybir.dt.float32
F8 = mybir.dt.float8e4
BF16 = mybir.dt.bfloat16
I16 = mybir.dt.int16

# Schraudolph exp constants for bf16 bit-trick: i16 = A*s + Bc, bitcast bf16
# exp(s/8) = 2^(s * 0.125 * log2(e)); bf16: i = 128*(log2(y) + 127)
SCH_A = 0.125 * 1.4426950408889634 * 128.0
SCH_B = 127.0 * 128.0 - 128.0 * 0.0579

# Pairs whose upper half (cols 512:1024) goes all-ScalarE instead of DVE,
# to balance ACT vs DVE load. DVE Schraudolph fraction = 14/32 ~ 44%.
ACT_ONLY_PAIRS = frozenset({4, 12})

LAST_EXEC_TIME_NS = None
LAST_RESULTS = None

_CACHED = None


def _core_segments(core):
    """Returns (headA, headB, (headC, qoff)) for this core."""
    start = core * ROWS_PER_CORE
    h = start // S
    if core % 2 == 0:
        return h, h + 1, (h + 2, 0)
    else:
        return h + 1, h + 2, (h, HALF)


def _build_graph(
    rows=ROWS_PER_CORE,
    npair=NPAIR,
    segs=None,
    nheads=3,
    num_devices=NCORES,
):
    """segs: list of (head_slot, q_row_base, n_q_rows)."""
    if segs is None:
        segs = [(0, 0, S), (1, S, S), (2, 2 * S, HALF)]
    nc = bacc.Bacc(
        "TRN2",
        target_bir_lowering=False,
        debug=False,
        num_devices=num_devices,
    )
    qT_ext = nc.dram_tensor("qT", [64, rows], F32, kind="ExternalInput").ap()
    kp_ext = nc.dram_tensor(
        "kp", [nheads, 128, npair * 128], F32, kind="ExternalInput"
    ).ap()
    vp_ext = nc.dram_tensor(
        "vp", [nheads, 128, 2 * npair * 65], F32, kind="ExternalInput"
    ).ap()
    out_ext = nc.dram_tensor("out", [rows, 64], F32, kind="ExternalOutput").ap()

    KW = npair * 128  # k columns per head
    VW = 2 * npair * 65  # v columns per head

    with tile.TileContext(nc) as tc:
        with (
            tc.tile_pool(name="persist", bufs=1) as persist,
            tc.tile_pool(name="stage", bufs=3) as stage,
            tc.tile_pool(name="pbuf", bufs=8) as pbuf,
            tc.tile_pool(name="work", bufs=2) as work,
            tc.tile_pool(name="qkpsum", bufs=2, space="PSUM") as qkpsum,
            tc.tile_pool(name="otpsum", bufs=2, space="PSUM") as otpsum,
        ):
            # ---- persistent operand tiles (split per head / per q-chunk
            # so the first segment's compute can start while later
            # heads are still loading) ----
            qc = min(2048, rows)
            nqc = rows // qc
            ident = persist.tile([65, 65], F32)
            make_identity(nc, ident[:])

            q8_t = [
                persist.tile([128, qc], F8, name=f"q8_{c}") for c in range(nqc)
            ]
            k8_t = [
                persist.tile([128, KW], F8, name=f"k8_{h}") for h in range(nheads)
            ]
            v8b_t = [
                persist.tile([128, VW], BF16, name=f"v8b_{h}")
                for h in range(nheads)
            ]

            def load_k(h, off, w, eng):
                st = stage.tile([128, 2080], F32, tag="stage", name=f"stk{h}{off}")
                nc.sync.dma_start(st[:, 0:w], kp_ext[h][:, off : off + w])
                eng.tensor_copy(k8_t[h][:, off : off + w], st[:, 0:w])

            def load_v(h, off, w, eng):
                st = stage.tile([128, 2080], F32, tag="stage", name=f"stv{h}{off}")
                nc.sync.dma_start(st[:, 0:w], vp_ext[h][:, off : off + w])
                v8f = work.tile([128, 2080], F8, tag="v8f", name=f"v8f{h}{off}")
                eng.tensor_copy(v8f[:, 0:w], st[:, 0:w])
                eng.tensor_copy(v8b_t[h][:, off : off + w], v8f[:, 0:w])

            def load_q(c, off, w, eng):
                st = stage.tile([128, 2080], F32, tag="stage", name=f"stq{c}{off}")
                nc.sync.dma_start(
                    st[0:64, 0:w], qT_ext[:, ts(c, qc)][:, off : off + w]
                )
                nc.sync.dma_start(
                    st[64:128, 0:w], qT_ext[:, ts(c, qc)][:, off : off + w]
                )
                eng.tensor_copy(q8_t[c][:, off : off + w], st[:, 0:w])

            # startup-critical pieces on DVE (fast, idle at kernel start);
            # everything that overlaps steady-state compute goes to GpSimd,
            # ordered by when the main loop first needs it.
            load_k(0, 0, 256, nc.vector)
            load_q(0, 0, 512, nc.vector)
            load_v(0, 0, 260, nc.vector)
            load_k(0, 256, KW - 256, nc.vector)
            load_v(0, 260, 780, nc.vector)
            load_v(0, 1040, VW - 1040, nc.gpsimd)
            load_q(0, 512, qc - 512, nc.gpsimd)
            if nqc > 1:
                load_q(1, 0, qc, nc.gpsimd)
            if nheads > 1:
                load_k(1, 0, KW, nc.gpsimd)
                load_v(1, 0, VW, nc.gpsimd)
            for c in range(2, nqc):
                load_q(c, 0, qc, nc.gpsimd)
            for h in range(2, nheads):
                load_k(h, 0, KW, nc.gpsimd)
                load_v(h, 0, VW, nc.gpsimd)

            # ---- main attention loops (software-pipelined) ----
            # Flat chunk list across segments: (head_slot, qtile, qo, qoff)
            chunks = []
            for slot, qbase, nq in segs:
                for chunk in range(nq // NQ):
                    qoff = qbase + chunk * NQ
                    chunks.append((slot, q8_t[qoff // qc], qoff % qc, qoff))

            def emit_qk_pair(slot, qtile, qo, p):
                # QK^T row-packed pair: A on partitions 0-63, B on
                # 64-127 (tile_position auto-derived from base partition)
                qk = qkpsum.tile(
                    [128, 2 * NQ], F32, tag="qk", bufs=3, name="qk"
                )
                kA = k8_t[slot][0:64, p * 128 : (p + 1) * 128]
                nc.tensor.matmul(
                    qk[:, 0:NQ], kA, qtile[0:64, qo : qo + NQ],
                    start=True, stop=True,
                )
                kB = k8_t[slot][64:128, p * 128 : (p + 1) * 128]
                nc.tensor.matmul(
                    qk[:, NQ : 2 * NQ], kB, qtile[64:128, qo : qo + NQ],
                    start=True, stop=True,
                )
                return qk

            def emit_exp(qk, p):
                # exp of the pair tile, split per half across engines:
                # ACT takes cols 0:512 (bank A), DVE cols 512:1024 (bank B)
                # so both engines read PSUM concurrently.
                pab = pbuf.tile([128, 2 * NQ], BF16, tag="p", name="pab")
                if p in ACT_ONLY_PAIRS:
                    nc.scalar.activation(
                        pab[:], qk[:],
                        mybir.ActivationFunctionType.Exp, scale=0.125,
                    )
                else:
                    nc.scalar.activation(
                        pab[:, 0:NQ], qk[:, 0:NQ],
                        mybir.ActivationFunctionType.Exp, scale=0.125,
                    )
                    nc.vector.tensor_scalar(
                        pab[:, NQ : 2 * NQ].bitcast(I16), qk[:, NQ : 2 * NQ],
                        SCH_A, SCH_B,
                        mybir.AluOpType.mult, mybir.AluOpType.add,
                    )
                return pab

            def emit_pv(slot, ot, pab, p, which):
                v = v8b_t[slot][
                    :, (2 * p + which) * 65 : (2 * p + which + 1) * 65
                ]
                nc.tensor.matmul(
                    ot[:], v, pab[:, which * NQ : (which + 1) * NQ],
                    start=(p == 0 and which == 0),
                    stop=(p == npair - 1 and which == 1),
                    skip_group_check=True,
                )

            def make_epilogue(ot, qoff):
                # two stages so the transpose burst doesn't monopolize the
                # shared ot/tr PSUM slots at chunk boundaries
                state = {}

                def epi_a():
                    ot_sb = work.tile([65, NQ], F32, tag="otsb", name="ot_sb")
                    nc.vector.tensor_copy(ot_sb[:], ot[:])
                    osb = work.tile([128, 4 * 64], F32, tag="osb", name="osb")
                    state["ot_sb"] = ot_sb
                    state["osb"] = osb
                    _tr_pair(0)

                def _tr_pair(base):
                    for t in (base, base + 1):
                        tr = otpsum.tile(
                            [128, 65], F32, tag="ot", bufs=2, name=f"tr{t}"
                        )
                        nc.tensor.transpose(
                            tr[:], state["ot_sb"][:, ts(t, 128)], ident[:]
                        )
                        rc = work.tile([128, 1], F32, tag="rc", name="rc")
                        nc.vector.reciprocal(rc[:], tr[:, 64:65])
                        nc.vector.tensor_scalar(
                            state["osb"][:, ts(t, 64)], tr[:, 0:64],
                            rc[:], None, mybir.AluOpType.mult,
                        )

                def epi_b():
                    _tr_pair(2)
                    nc.sync.dma_start(
                        out_ext[qoff : qoff + NQ, :].rearrange(
                            "(b p) d -> p b d", p=128
                        ),
                        state["osb"][:].rearrange("p (b d) -> p b d", d=64),
                    )

                return epi_a, epi_b

            # Flat (chunk, pair) stream with a 2-pair QK lookahead that
            # crosses chunk boundaries, so exp always has >= 2 pairs of
            # lead over the PV that consumes it.
            nchunks = len(chunks)
            npairs_total = nchunks * npair

            def pair_at(i):
                slot, qtile, qo, _ = chunks[i // npair]
                return slot, qtile, qo, i % npair

            ots = {}
            qks = {}
            pending_epi = None
            LOOKAHEAD = 2
            for i in range(min(LOOKAHEAD, npairs_total)):
                qks[i] = emit_qk_pair(*pair_at(i))
            for i in range(npairs_total):
                c, p = divmod(i, npair)
                slot, qtile, qo, qoff = chunks[c]
                if p == 0:
                    ots[c] = otpsum.tile(
                        [65, NQ], F32, tag="ot", bufs=2, name="ot"
                    )
                if i + LOOKAHEAD < npairs_total:
                    qks[i + LOOKAHEAD] = emit_qk_pair(*pair_at(i + LOOKAHEAD))
                if pending_epi is not None:
                    if p == 3:
                        pending_epi[0]()
                    elif p == 7:
                        pending_epi[1]()
                        pending_epi = None
                pab = emit_exp(qks.pop(i), p)
                emit_pv(slot, ots[c], pab, p, 0)
                emit_pv(slot, ots[c], pab, p, 1)
                if p == npair - 1:
                    if pending_epi is not None:
                        pending_epi[0]()
                        pending_epi[1]()
                    pending_epi = make_epilogue(ots.pop(c), qoff)
            pending_epi[0]()
            pending_epi[1]()

    nc.compile()
    return nc


def _prep_core_inputs(core, q, k, v):
    hA, hB, (hC, qoff) = _core_segments(core)
    qT = np.empty((64, ROWS_PER_CORE), np.float32)
    qT[:, 0:S] = q[hA].T
    qT[:, S : 2 * S] = q[hB].T
    qT[:, 2 * S :] = q[hC, qoff : qoff + HALF].T

    kp = np.empty((3, 128, NPAIR * 128), np.float32)
    vp = np.empty((3, 128, 32 * 65), np.float32)
    for slot, h in enumerate((hA, hB, hC)):
        kt = np.ascontiguousarray(k[h].T).reshape(64, 32, 128)
        kp[slot, 0:64] = kt[:, 0::2, :].reshape(64, NPAIR * 128)
        kp[slot, 64:128] = kt[:, 1::2, :].reshape(64, NPAIR * 128)
        vb = v[h].reshape(32, 128, 64).transpose(1, 0, 2)  # [128, 32, 64]
        vpk = np.concatenate(
            [vb, np.ones((128, 32, 1), np.float32)], axis=2
        )  # [128, 32, 65]
        vp[slot] = vpk.reshape(128, 32 * 65)
    return {"qT": np.ascontiguousarray(qT), "kp": kp, "vp": vp}


def kernel(q, k, v):
    global LAST_EXEC_TIME_NS, LAST_RESULTS, _CACHED
    q = np.asarray(q, np.float32)
    k = np.asarray(k, np.float32)
    v = np.asarray(v, np.float32)

    if _CACHED is None:
        _CACHED = _build_graph()
    nc = _CACHED

    in_maps = [_prep_core_inputs(i, q, k, v) for i in range(NCORES)]

    trace = os.environ.get("KERNEL_TRACE", "0") == "1"
    kwargs = {}
    if trace:
        kwargs = dict(trace=True, trace_cores=[0])
    res = run_bass_kernel_spmd(nc, in_maps, core_ids=list(range(NCORES)), **kwargs)
    LAST_RESULTS = res
    LAST_EXEC_TIME_NS = res.exec_time_ns

    out = np.empty((B, S, D), np.float32)
    for core in range(NCORES):
        o = res.results[core]["out"]
        hA, hB, (hC, qoff) = _core_segments(core)
        out[hA] = o[0:S]
        out[hB] = o[S : 2 * S]
        out[hC, qoff : qoff + HALF] = o[2 * S :]
    return out


# revision 23
# speedup vs baseline: 1.1928x; 1.1928x over previous
"""Trainium2 Bass kernel: fp8-emulated attention, 20 heads x 4096 x 64.

Sharding: flattened (head, q) rows split evenly across 8 cores ->
2.5 heads per core (2 full-head segments + 1 half-head segment each,
identical SPMD graph; per-core in_maps differ only in data).

Per-core algorithm (S.T layout flash-style, no max subtraction -- scores
are bounded ~|s|<7 so fp32 exp never overflows):
  q8 = fp8(q) replicated on both partition halves, d-major [128, 10240]
  k8 = fp8(k) pair-packed [128, 16*128] per head (even kv-blocks on
       partitions 0-63, odd on 64-127) -> row-packed (tile_position)
       pairs of QK^T matmuls, K=64 contraction each.
  S.T block [128 kv, 512 q] in PSUM -> exp split per half: ScalarE does
  cols 0:512 (exact exp, scale=1/8 fused), VectorE does cols 512:1024
  (Schraudolph int16 bit-trick -> bf16) on most pairs; a few pairs go
  all-ScalarE to balance engine load. The two halves live in different
  PSUM banks so ACT+DVE read PSUM concurrently (this split beats
  whole-pair alternation: it halves per-pair exp latency).
  PV: O.T[65, 512] += [V_blk | ones].T @ P.T_blk accumulated over 32
  kv-blocks in PSUM; row 64 = softmax denominators.
  Epilogue (split across two pair-slots of the next chunk): copy to
  SBUF, PE-transpose [65,128] tiles -> [128,65], DVE reciprocal of the
  denominator + ScalarE Copy-with-AP-scale -> out rows [q, 64] -> DMA.
The (chunk, pair) stream is a single flat pipeline with a 2-pair QK
lookahead that crosses chunk boundaries. Bulk input casts (fp32 ->
fp8/bf16) run on GpSimd ordered by first use; the startup-critical
first pieces are cast on DVE so the first QK starts ~13us in.
"""

import os

import numpy as np

import concourse.bass as bass
import concourse.tile as tile
from concourse import bacc, mybir
from concourse.bass import ts
from concourse.bass_utils import run_bass_kernel_spmd
from concourse.masks import make_identity

B, S, D = 20, 4096, 64
NCORES = 8
ROWS_PER_CORE = B * S // NCORES  # 10240
HALF = S // 2  # 2048
NQ = 512  # q columns per chunk (one PSUM bank of fp32)
NPAIR = 16  # kv-block pairs per head (32 blocks of 128)

F32 = mybir.dt.float32
F8 = mybir.dt.float8e4
BF16 = mybir.dt.bfloat16
I16 = mybir.dt.int16

# Schraudolph exp constants for bf16 bit-trick: i16 = A*s + Bc, bitcast bf16
# exp(s/8) = 2^(s * 0.125 * log2(e)); bf16: i = 128*(log2(y) + 127)
SCH_A = 0.125 * 1.4426950408889634 * 128.0
SCH_B = 127.0 * 128.0 - 128.0 * 0.0579

# Pairs whose upper half (cols 512:1024) goes all-ScalarE instead of DVE,
# to balance ACT vs DVE load. DVE Schraudolph fraction = 13/32 ~ 41%.
ACT_ONLY_PAIRS = frozenset({4, 9, 14})

LAST_EXEC_TIME_NS = None
LAST_RESULTS = None

_CACHED = None


def _core_segments(core):
    """Returns (headA, headB, (headC, qoff)) for this core."""
    start = core * ROWS_PER_CORE
    h = start // S
    if core % 2 == 0:
        return h, h + 1, (h + 2, 0)
    else:
        return h + 1, h + 2, (h, HALF)


def _build_graph(
    rows=ROWS_PER_CORE,
    npair=NPAIR,
    segs=None,
    nheads=3,
    num_devices=NCORES,
):
    """segs: list of (head_slot, q_row_base, n_q_rows)."""
    if segs is None:
        segs = [(0, 0, S), (1, S, S), (2, 2 * S, HALF)]
    nc = bacc.Bacc(
        "TRN2",
        target_bir_lowering=False,
        debug=False,
        num_devices=num_devices,
    )
    qT_ext = nc.dram_tensor("qT", [64, rows], F32, kind="ExternalInput").ap()
    kp_ext = nc.dram_tensor(
        "kp", [nheads, 128, npair * 128], F32, kind="ExternalInput"
    ).ap()
    vp_ext = nc.dram_tensor(
        "vp", [nheads, 128, 2 * npair * 65], F32, kind="ExternalInput"
    ).ap()
    out_ext = nc.dram_tensor("out", [rows, 64], F32, kind="ExternalOutput").ap()

    KW = npair * 128  # k columns per head
    VW = 2 * npair * 65  # v columns per head

    with tile.TileContext(nc) as tc:
        with (
            tc.tile_pool(name="persist", bufs=1) as persist,
            tc.tile_pool(name="stage", bufs=3) as stage,
            tc.tile_pool(name="pbuf", bufs=8) as pbuf,
            tc.tile_pool(name="work", bufs=2) as work,
            tc.tile_pool(name="qkpsum", bufs=2, space="PSUM") as qkpsum,
            tc.tile_pool(name="otpsum", bufs=2, space="PSUM") as otpsum,
        ):
            # ---- persistent operand tiles (split per head / per q-chunk
            # so the first segment's compute can start while later
            # heads are still loading) ----
            qc = min(2048, rows)
            nqc = rows // qc
            ident = persist.tile([65, 65], F32)
            make_identity(nc, ident[:])

            q8_t = [
                persist.tile([128, qc], F8, name=f"q8_{c}") for c in range(nqc)
            ]
            k8_t = [
                persist.tile([128, KW], F8, name=f"k8_{h}") for h in range(nheads)
            ]
            v8b_t = [
                persist.tile([128, VW], BF16, name=f"v8b_{h}")
                for h in range(nheads)
            ]

            def load_k(h, off, w, eng):
                st = stage.tile([128, 2080], F32, tag="stage", name=f"stk{h}{off}")
                nc.sync.dma_start(st[:, 0:w], kp_ext[h][:, off : off + w])
                eng.tensor_copy(k8_t[h][:, off : off + w], st[:, 0:w])

            def load_v(h, off, w, eng):
                st = stage.tile([128, 2080], F32, tag="stage", name=f"stv{h}{off}")
                nc.sync.dma_start(st[:, 0:w], vp_ext[h][:, off : off + w])
                v8f = work.tile([128, 2080], F8, tag="v8f", name=f"v8f{h}{off}")
                eng.tensor_copy(v8f[:, 0:w], st[:, 0:w])
                eng.tensor_copy(v8b_t[h][:, off : off + w], v8f[:, 0:w])

            def load_q(c, off, w, eng):
                st = stage.tile([128, 2080], F32, tag="stage", name=f"stq{c}{off}")
                nc.sync.dma_start(
                    st[0:64, 0:w], qT_ext[:, ts(c, qc)][:, off : off + w]
                )
                nc.sync.dma_start(
                    st[64:128, 0:w], qT_ext[:, ts(c, qc)][:, off : off + w]
                )
                eng.tensor_copy(q8_t[c][:, off : off + w], st[:, 0:w])

            # startup-critical pieces on DVE (fast, idle at kernel start);
            # everything that overlaps steady-state compute goes to GpSimd,
            # ordered by when the main loop first needs it.
            load_k(0, 0, 256, nc.vector)
            load_q(0, 0, 512, nc.vector)
            load_v(0, 0, 260, nc.vector)
            load_k(0, 256, KW - 256, nc.vector)
            load_v(0, 260, 780, nc.vector)
            load_v(0, 1040, VW - 1040, nc.gpsimd)
            load_q(0, 512, qc - 512, nc.gpsimd)
            if nqc > 1:
                load_q(1, 0, qc, nc.gpsimd)
            if nheads > 1:
                load_k(1, 0, KW, nc.gpsimd)
                load_v(1, 0, VW, nc.gpsimd)
            for c in range(2, nqc):
                load_q(c, 0, qc, nc.gpsimd)
            for h in range(2, nheads):
                load_k(h, 0, KW, nc.gpsimd)
                load_v(h, 0, VW, nc.gpsimd)

            # ---- main attention loops (software-pipelined) ----
            # Flat chunk list across segments: (head_slot, qtile, qo, qoff)
            chunks = []
            for slot, qbase, nq in segs:
                for chunk in range(nq // NQ):
                    qoff = qbase + chunk * NQ
                    chunks.append((slot, q8_t[qoff // qc], qoff % qc, qoff))

            def emit_qk_pair(slot, qtile, qo, p):
                # QK^T row-packed pair: A on partitions 0-63, B on
                # 64-127 (tile_position auto-derived from base partition)
                qk = qkpsum.tile(
                    [128, 2 * NQ], F32, tag="qk", bufs=3, name="qk"
                )
                kA = k8_t[slot][0:64, p * 128 : (p + 1) * 128]
                nc.tensor.matmul(
                    qk[:, 0:NQ], kA, qtile[0:64, qo : qo + NQ],
                    start=True, stop=True,
                )
                kB = k8_t[slot][64:128, p * 128 : (p + 1) * 128]
                nc.tensor.matmul(
                    qk[:, NQ : 2 * NQ], kB, qtile[64:128, qo : qo + NQ],
                    start=True, stop=True,
                )
                return qk

            def emit_exp(qk, p):
                # exp of the pair tile, split per half across engines:
                # ACT takes cols 0:512 (bank A), DVE cols 512:1024 (bank B)
                # so both engines read PSUM concurrently.
                pab = pbuf.tile([128, 2 * NQ], BF16, tag="p", name="pab")
                if p in ACT_ONLY_PAIRS:
                    nc.scalar.activation(
                        pab[:], qk[:],
                        mybir.ActivationFunctionType.Exp, scale=0.125,
                    )
                else:
                    nc.scalar.activation(
                        pab[:, 0:NQ], qk[:, 0:NQ],
                        mybir.ActivationFunctionType.Exp, scale=0.125,
                    )
                    nc.vector.tensor_scalar(
                        pab[:, NQ : 2 * NQ].bitcast(I16), qk[:, NQ : 2 * NQ],
                        SCH_A, SCH_B,
                        mybir.AluOpType.mult, mybir.AluOpType.add,
                    )
                return pab

            def emit_pv(slot, ot, pab, p, which):
                v = v8b_t[slot][
                    :, (2 * p + which) * 65 : (2 * p + which + 1) * 65
                ]
                nc.tensor.matmul(
                    ot[:], v, pab[:, which * NQ : (which + 1) * NQ],
                    start=(p == 0 and which == 0),
                    stop=(p == npair - 1 and which == 1),
                    skip_group_check=True,
                )

            def make_epilogue(ot, qoff):
                # two stages so the transpose burst doesn't monopolize the
                # shared ot/tr PSUM slots at chunk boundaries
                state = {}

                def epi_a():
                    ot_sb = work.tile([65, NQ], F32, tag="otsb", name="ot_sb")
                    nc.vector.tensor_copy(ot_sb[:], ot[:])
                    osb = work.tile([128, 4 * 64], F32, tag="osb", name="osb")
                    state["ot_sb"] = ot_sb
                    state["osb"] = osb
                    _tr_pair(0)

                def _tr_pair(base):
                    for t in (base, base + 1):
                        tr = otpsum.tile(
                            [128, 65], F32, tag="ot", bufs=2, name=f"tr{t}"
                        )
                        nc.tensor.transpose(
                            tr[:], state["ot_sb"][:, ts(t, 128)], ident[:]
                        )
                        rc = work.tile([128, 1], F32, tag="rc", name="rc")
                        nc.vector.reciprocal(rc[:], tr[:, 64:65])
                        nc.vector.tensor_scalar(
                            state["osb"][:, ts(t, 64)], tr[:, 0:64],
                            rc[:], None, mybir.AluOpType.mult,
                        )

                def epi_b():
                    _tr_pair(2)
                    nc.sync.dma_start(
                        out_ext[qoff : qoff + NQ, :].rearrange(
                            "(b p) d -> p b d", p=128
                        ),
                        state["osb"][:].rearrange("p (b d) -> p b d", d=64),
                    )

                return epi_a, epi_b

            # Flat (chunk, pair) stream with a 2-pair QK lookahead that
            # crosses chunk boundaries, so exp always has >= 2 pairs of
            # lead over the PV that consumes it.
            nchunks = len(chunks)
            npairs_total = nchunks * npair

            def pair_at(i):
                slot, qtile, qo, _ = chunks[i // npair]
                return slot, qtile, qo, i % npair

            ots = {}
            qks = {}
            pending_epi = None
            LOOKAHEAD = 2
            for i in range(min(LOOKAHEAD, npairs_total)):
                qks[i] = emit_qk_pair(*pair_at(i))
            for i in range(npairs_total):
                c, p = divmod(i, npair)
                slot, qtile, qo, qoff = chunks[c]
                if p == 0:
                    ots[c] = otpsum.tile(
                        [65, NQ], F32, tag="ot", bufs=2, name="ot"
                    )
                if i + LOOKAHEAD < npairs_total:
                    qks[i + LOOKAHEAD] = emit_qk_pair(*pair_at(i + LOOKAHEAD))
                if pending_epi is not None:
                    if p == 3:
                        pending_epi[0]()
                    elif p == 7:
                        pending_epi[1]()
                        pending_epi = None
                pab = emit_exp(qks.pop(i), p)
                emit_pv(slot, ots[c], pab, p, 0)
                emit_pv(slot, ots[c], pab, p, 1)
                if p == npair - 1:
                    if pending_epi is not None:
                        pending_epi[0]()
                        pending_epi[1]()
                    pending_epi = make_epilogue(ots.pop(c), qoff)
            pending_epi[0]()
            pending_epi[1]()

    nc.compile()
    return nc


def _prep_core_inputs(core, q, k, v):
    hA, hB, (hC, qoff) = _core_segments(core)
    qT = np.empty((64, ROWS_PER_CORE), np.float32)
    qT[:, 0:S] = q[hA].T
    qT[:, S : 2 * S] = q[hB].T
    qT[:, 2 * S :] = q[hC, qoff : qoff + HALF].T

    kp = np.empty((3, 128, NPAIR * 128), np.float32)
    vp = np.empty((3, 128, 32 * 65), np.float32)
    for slot, h in enumerate((hA, hB, hC)):
        kt = np.ascontiguousarray(k[h].T).reshape(64, 32, 128)
        kp[slot, 0:64] = kt[:, 0::2, :].reshape(64, NPAIR * 128)
        kp[slot, 64:128] = kt[:, 1::2, :].reshape(64, NPAIR * 128)
        vb = v[h].reshape(32, 128, 64).transpose(1, 0, 2)  # [128, 32, 64]
        vpk = np.concatenate(
            [vb, np.ones((128, 32, 1), np.float32)], axis=2
        )  # [128, 32, 65]
        vp[slot] = vpk.reshape(128, 32 * 65)
    return {"qT": np.ascontiguousarray(qT), "kp": kp, "vp": vp}


def kernel(q, k, v):
    global LAST_EXEC_TIME_NS, LAST_RESULTS, _CACHED
    q = np.asarray(q, np.float32)
    k = np.asarray(k, np.float32)
    v = np.asarray(v, np.float32)

    if _CACHED is None:
        _CACHED = _build_graph()
    nc = _CACHED

    in_maps = [_prep_core_inputs(i, q, k, v) for i in range(NCORES)]

    trace = os.environ.get("KERNEL_TRACE", "0") == "1"
    kwargs = {}
    if trace:
        kwargs = dict(trace=True, trace_cores=[0])
    res = run_bass_kernel_spmd(nc, in_maps, core_ids=list(range(NCORES)), **kwargs)
    LAST_RESULTS = res
    LAST_EXEC_TIME_NS = res.exec_time_ns

    out = np.empty((B, S, D), np.float32)
    for core in range(NCORES):
        o = res.results[core]["out"]
        hA, hB, (hC, qoff) = _core_segments(core)
        out[hA] = o[0:S]
        out[hB] = o[S : 2 * S]
        out[hC, qoff : qoff + HALF] = o[2 * S :]
    return out
